# revision 1
# baseline (speedup 1.0000x reference)
"""Trainium2 Bass kernel for nn_FCLTCQNetwork (LTC recurrent Q-network).

Strategy (8 NeuronCores, data-parallel over batch; B=4096 -> 512/core):
  - Transposed layout: neuron/feature dims on SBUF partitions, batch on the
    free dim. Each core's 512-batch is further split into 2 independent
    256-wide streams so one stream's serial update chain (reciprocal on DVE)
    hides under the other stream's matmul/sigmoid work.
  - Each sigmoid block (sensory or one of the 6 ODE unfolds) is 64x64
    sigmoids per batch element, tiled as 32 chunks of (i,u) pairs on the 128
    partitions. A per-chunk "expansion" matmul (float32r, diag(sigma) one-hot
    + hi/lo bias rows against constant-1 rhs rows) computes the full
    pre-activation in PSUM; one ACT Sigmoid op covers 4 chunks (FD=1024);
    a per-chunk "reduction" matmul computes sum_i w*erev*sig (numerator) and
    sum_i w*sig (denominator) for 2 u's per chunk, PSUM-accumulated over all
    32 chunks.
  - Update (DVE + GpSimd): num = S1 + w_num_s + cm_t*v + gl*vleak;
    den = S2 + w_den_s + cm_t + gl + eps; v' = num * reciprocal(den).
  - T=128 timesteps fully unrolled (For_i + DMA is broken in this walrus
    snapshot: back-edge drains overflow the ISA sync-wait slots).
  - split_excess_waits(): post-pass hoisting >1 sync-waits per instruction
    onto preceding NoOps (TPB encodings only fit 1).
"""

import sys
import types

import numpy as np

# ---- NTFF profile hook shim (antenv.axon_hooks is absent in this image) ----
try:
    import trn_agent_boot.trn_boot as _tb

    _hook = _tb._ntff_profile_via_ctypes("/opt/axon/libaxon_pjrt.so")
    if "antenv.axon_hooks" not in sys.modules:
        _m = types.ModuleType("antenv.axon_hooks")
        _m.get_axon_ntff_profile_hook = lambda: _hook
        sys.modules["antenv.axon_hooks"] = _m
except Exception:
    pass

import concourse.bass as bass
import concourse.mybir as mybir
import concourse.tile as tile
import concourse.tile_sem_assignment as _tsa

# Cap the HWDGE queue round-robin so Drain instructions stay encodable
# (walrus emits one sync-wait per active DMA queue; >~4 overflows CTRL).
_tsa.NUM_HWDGE_SEMS = 2

F32 = mybir.dt.float32
F32R = mybir.dt.float32r
AF = mybir.ActivationFunctionType
ALU = mybir.AluOpType

B, T, D, U, A, QH = 4096, 128, 64, 64, 8, 64
UNFOLDS = 6
EPS = 1e-8
N_CORES = 8
B_SH = B // N_CORES          # 512
BS = B_SH // 2               # 256 per stream
NCH = 32
GROUPS4 = [list(range(g, g + 4)) for g in range(0, NCH, 4)]


def _tf32_round(x):
    x = np.asarray(x, np.float32)
    xi = x.view(np.uint32).astype(np.uint64)
    lsb = (xi >> 13) & 1
    return ((xi + 0xFFF + lsb) & 0xFFFFE000).astype(np.uint32).view(np.float32)


def _hi_lo(x):
    hi = _tf32_round(x)
    lo = _tf32_round(np.asarray(x, np.float32) - hi)
    return hi, lo


def _softplus_f32(x):
    return np.log1p(np.exp(np.asarray(x, np.float32))).astype(np.float32)


def _precompute(params):
    ss = np.asarray(params["sensory_sigma"], np.float32)
    smu = np.asarray(params["sensory_mu"], np.float32)
    sw_p = _softplus_f32(params["sensory_w"])
    serev = np.asarray(params["sensory_erev"], np.float32)
    sig = np.asarray(params["sigma"], np.float32)
    mu = np.asarray(params["mu"], np.float32)
    w_p = _softplus_f32(params["w"])
    erev = np.asarray(params["erev"], np.float32)
    gl = _softplus_f32(params["gleak"])
    vleak = np.asarray(params["vleak"], np.float32)
    cm_t = _softplus_f32(params["cm"]) * np.float32(UNFOLDS)
    in_w = np.asarray(params["input_w"], np.float32)
    in_b = np.asarray(params["input_b"], np.float32)
    W1 = np.asarray(params["W1"], np.float32)
    b1 = np.asarray(params["b1"], np.float32)
    W2 = np.asarray(params["W2"], np.float32)
    b2 = np.asarray(params["b2"], np.float32)

    def build_exp(scale_iu, bias_iu):
        out = np.zeros((NCH, 66, 128), np.float32)
        for c in range(NCH):
            for du in range(2):
                u = 2 * c + du
                cols = slice(du * 64, du * 64 + 64)
                out[c, 0:64, cols] = np.diag(scale_iu[:, u])
                bh, bl = _hi_lo(bias_iu[:, u])
                out[c, 64, cols] = bh
                out[c, 65, cols] = bl
        return out

    def build_red(wE_iu, wp_iu):
        out = np.zeros((NCH, 128, 128), np.float32)
        for c in range(NCH):
            for du in range(2):
                u = 2 * c + du
                rows = slice(du * 64, du * 64 + 64)
                out[c, rows, u] = wE_iu[:, u]
                out[c, rows, 64 + u] = wp_iu[:, u]
        return out

    pp = np.zeros((128, 3), np.float32)
    pp[0:64, 0] = cm_t
    pp[0:64, 1] = gl * vleak
    pp[64:128, 2] = cm_t + gl + np.float32(EPS)

    W1aug = np.zeros((66, QH), np.float32)
    W1aug[0:64] = W1
    W1aug[64], W1aug[65] = _hi_lo(b1)
    W2aug = np.zeros((66, A), np.float32)
    W2aug[0:64] = W2
    W2aug[64], W2aug[65] = _hi_lo(b2)

    init = np.zeros((66, B_SH), np.float32)
    init[64:66] = 1.0

    return {
        "SEN_EXP": build_exp(ss * in_w[:, None], ss * (in_b[:, None] - smu)
                             ).reshape(NCH * 66, 128),
        "REC_EXP": build_exp(sig, -sig * mu).reshape(NCH * 66, 128),
        "SEN_RED": build_red(sw_p * serev, sw_p).reshape(NCH * 128, 128),
        "REC_RED": build_red(w_p * erev, w_p).reshape(NCH * 128, 128),
        "PP": pp,
        "W1AUG": W1aug,
        "W2AUG": W2aug,
        "INIT": init,
    }


def split_excess_waits(nc, cap=1):
    n_split = 0
    for bb in nc.main_func.blocks:
        new_insts = []
        for inst in bb.instructions:
            si = getattr(inst, "sync_info", None)
            if si is not None and len(si.on_wait) > cap:
                waits = list(si.on_wait)
                excess, keep = waits[:-cap], waits[-cap:]
                for i, w in enumerate(excess):
                    nop = mybir.InstNoOp(
                        name=f"{inst.name}-wsplit{i}",
                        opcode="NoOp",
                        engine=inst.engine,
                        ins=[],
                        outs=[],
                        sync_info=mybir.SyncInfo(on_wait=[w], on_update=[]),
                    )
                    new_insts.append(nop)
                    n_split += 1
                inst.sync_info = mybir.SyncInfo(
                    on_wait=keep, on_update=list(si.on_update)
                )
            new_insts.append(inst)
        bb.instructions[:] = new_insts
    return n_split


def build_kernel(T_steps=T):
    nc = bass.Bass(target_bir_lowering=False)
    DT = F32R

    obsT = nc.dram_tensor("OBST", [T_steps, 64, B_SH], DT, kind="ExternalInput")
    d_sen_exp = nc.dram_tensor("SEN_EXP", [NCH * 66, 128], DT, kind="ExternalInput")
    d_rec_exp = nc.dram_tensor("REC_EXP", [NCH * 66, 128], DT, kind="ExternalInput")
    d_sen_red = nc.dram_tensor("SEN_RED", [NCH * 128, 128], DT, kind="ExternalInput")
    d_rec_red = nc.dram_tensor("REC_RED", [NCH * 128, 128], DT, kind="ExternalInput")
    d_pp = nc.dram_tensor("PP", [128, 3], F32, kind="ExternalInput")
    d_w1 = nc.dram_tensor("W1AUG", [66, QH], DT, kind="ExternalInput")
    d_w2 = nc.dram_tensor("W2AUG", [66, A], DT, kind="ExternalInput")
    d_init = nc.dram_tensor("INIT", [66, B_SH], DT, kind="ExternalInput")
    d_q = nc.dram_tensor("QT", [A, B_SH], F32, kind="ExternalOutput")
    d_h = nc.dram_tensor("HT", [64, B_SH], F32, kind="ExternalOutput")

    with tile.TileContext(nc) as tc:
        with (
            tc.tile_pool(name="singles", bufs=1) as singles,
            tc.tile_pool(name="sigs", bufs=4) as sigs,
            tc.tile_pool(name="upd", bufs=2) as upd,
            tc.tile_pool(name="pexp", bufs=2, space="PSUM") as pexp,
            tc.tile_pool(name="pred", bufs=1, space="PSUM") as pred,
            tc.tile_pool(name="psen", bufs=1, space="PSUM") as psen,
        ):
            sen_exp = singles.tile([66, NCH, 128], DT)
            rec_exp = singles.tile([66, NCH, 128], DT)
            sen_red = singles.tile([128, NCH, 128], DT)
            rec_red = singles.tile([128, NCH, 128], DT)
            pp = singles.tile([128, 3], F32)
            w1 = singles.tile([66, QH], DT)
            w2 = singles.tile([66, A], DT)
            nc.sync.dma_start(sen_exp[:], d_sen_exp[:].rearrange("(c k) m -> k c m", k=66))
            nc.sync.dma_start(rec_exp[:], d_rec_exp[:].rearrange("(c k) m -> k c m", k=66))
            nc.sync.dma_start(sen_red[:], d_sen_red[:].rearrange("(c k) m -> k c m", k=128))
            nc.sync.dma_start(rec_red[:], d_rec_red[:].rearrange("(c k) m -> k c m", k=128))
            nc.sync.dma_start(pp[:], d_pp[:])
            nc.sync.dma_start(w1[:], d_w1[:])
            nc.sync.dma_start(w2[:], d_w2[:])

            vX = singles.tile([66, B_SH], DT)
            nc.sync.dma_start(vX[:], d_init[:])
            NXT = 3
            xts = [singles.tile([66, B_SH], DT, tag=f"xt{i}", name=f"xt{i}")
                   for i in range(NXT)]
            for x in xts:
                nc.sync.dma_start(x[64:66, :], d_init[64:66, :])
            wns = [singles.tile([128, B_SH], F32, tag=f"wns{i}", name=f"wns{i}")
                   for i in range(2)]

            def sl(s):
                return slice(s * BS, (s + 1) * BS)

            def emit_group(exp_w, red_w, state, s, accum_psum, gi):
                grp = GROUPS4[gi]
                pg = pexp.tile([128, 4 * BS], F32, tag="pg")
                for j, c in enumerate(grp):
                    nc.tensor.matmul(
                        pg[:, j * BS:(j + 1) * BS],
                        exp_w[:, c, :], state[:, sl(s)],
                        start=True, stop=True,
                    )
                st = sigs.tile([128, 4 * BS], DT, tag="st")
                nc.scalar.activation(st[:], pg[:], AF.Sigmoid)
                for j, c in enumerate(grp):
                    nc.tensor.matmul(
                        accum_psum[:],
                        red_w[:, c, :], st[:, j * BS:(j + 1) * BS],
                        start=(c == 0), stop=(c == NCH - 1),
                    )

            def emit_update(s, pr, par):
                t1 = upd.tile([64, BS], F32, tag=f"t1{s}")
                nc.gpsimd.tensor_scalar(
                    t1[:], vX[0:64, sl(s)].bitcast(F32),
                    pp[0:64, 0:1], pp[0:64, 1:2], ALU.mult, ALU.add,
                )
                nd = upd.tile([128, BS], F32, tag=f"nd{s}")
                nc.vector.tensor_add(nd[:], pr[:], wns[par][:, sl(s)])
                rec = upd.tile([64, BS], F32, tag=f"rec{s}")
                nc.vector.reciprocal(rec[:], nd[64:128, :])
                num = upd.tile([64, BS], F32, tag=f"num{s}")
                nc.gpsimd.tensor_add(num[:], nd[0:64, :], t1[:])
                nc.vector.tensor_mul(vX[0:64, sl(s)], num[:], rec[:])

            NGRP = len(GROUPS4)                       # 8
            SEN_UNITS = [(s, gi) for gi in range(NGRP) for s in (0, 1)]  # 16
            SEN_SCHED = [3, 3, 3, 3, 2, 2]

            nc.sync.dma_start(xts[0][0:64, :], obsT[0])
            ps0 = [psen.tile([128, BS], F32, tag=f"ps{s}", name=f"ps0_{s}")
                   for s in (0, 1)]
            for s, gi in SEN_UNITS:
                emit_group(sen_exp, sen_red, xts[0], s, ps0[s], gi)
            for s in (0, 1):
                nc.vector.tensor_scalar_add(wns[0][:, sl(s)], ps0[s][:], pp[:, 2:3])

            for t in range(T_steps):
                par = t % 2
                have_next = t + 1 < T_steps
                if have_next:
                    nc.sync.dma_start(xts[(t + 1) % NXT][0:64, :], obsT[t + 1])
                    ps_next = [psen.tile([128, BS], F32, tag=f"ps{s}",
                                         name=f"ps{t+1}_{s}") for s in (0, 1)]
                udone = 0
                for k in range(UNFOLDS):
                    prs = [pred.tile([128, BS], F32, tag=f"pr{s}",
                                     name=f"pr{t}_{k}_{s}") for s in (0, 1)]
                    for s in (0, 1):
                        for gi in range(NGRP):
                            emit_group(rec_exp, rec_red, vX, s, prs[s], gi)
                        emit_update(s, prs[s], par)
                    if have_next:
                        for _ in range(SEN_SCHED[k]):
                            s, gi = SEN_UNITS[udone]
                            emit_group(sen_exp, sen_red, xts[(t + 1) % NXT], s,
                                       ps_next[s], gi)
                            udone += 1
                if have_next:
                    for s in (0, 1):
                        nc.vector.tensor_scalar_add(
                            wns[(t + 1) % 2][:, sl(s)], ps_next[s][:], pp[:, 2:3]
                        )

            ph1 = psen.tile([64, B_SH], F32, tag="ps0")
            nc.tensor.matmul(ph1[:], w1[:], vX[:], start=True, stop=True)
            h1 = singles.tile([66, B_SH], DT)
            nc.sync.dma_start(h1[64:66, :], d_init[64:66, :])
            nc.scalar.activation(h1[0:64, :], ph1[:], AF.Relu)
            pq = pred.tile([A, B_SH], F32, tag="pr0")
            nc.tensor.matmul(pq[:], w2[:], h1[:], start=True, stop=True)
            qt = singles.tile([A, B_SH], F32)
            nc.vector.tensor_copy(qt[:], pq[:])
            nc.sync.dma_start(d_q[:], qt[:])
            nc.sync.dma_start(d_h[:], vX[0:64, :].bitcast(F32))

    split_excess_waits(nc)
    return nc


_CACHE = {}


def _get_kernel():
    if "nc" not in _CACHE:
        _CACHE["nc"] = build_kernel(T)
    return _CACHE["nc"]


def kernel(**inputs):
    """Full (unsharded) inputs -> (q [B, A], hidden [B, U]), float32."""
    from concourse.bass_utils import run_bass_kernel_spmd

    obs = np.asarray(inputs["obs"], np.float32)
    dram = _precompute(inputs)

    in_maps = []
    for c in range(N_CORES):
        shard = obs[c * B_SH:(c + 1) * B_SH]                  # (512, T, 64)
        obsT = np.ascontiguousarray(shard.transpose(1, 2, 0))  # (T, 64, 512)
        m = {"OBST": obsT}
        m.update(dram)
        in_maps.append(m)

    nc = _get_kernel()
    res = run_bass_kernel_spmd(nc, in_maps, list(range(N_CORES)), trace=False)

    q = np.zeros((B, A), np.float32)
    h = np.zeros((B, U), np.float32)
    for c in range(N_CORES):
        q[c * B_SH:(c + 1) * B_SH] = res.results[c]["QT"].T
        h[c * B_SH:(c + 1) * B_SH] = res.results[c]["HT"].T
    return q, h
